# revision 7
# baseline (speedup 1.0000x reference)
"""BiGCN (two fused GCNConv + graph mean-pool + FC + log_softmax) on 8 trn2 cores.

Strategy (graph/data parallel, partitioned by destination node range):
  - core c owns nodes [c*NSH, (c+1)*NSH) as edge destinations
  - host sorts/pads edges into per-(dst-tile, table-bank) chunks of 128
  - device: degree count via one-hot matmul -> dinv = rsqrt(deg+1)
            Hn = (x @ [W_td | W_bu]) * dinv  (bf16), AllGather -> full table
            dma_gather Hn[src] rows, one-hot matmul scatter-add into PSUM
            out[d] = dinv[d]*(sum + Hn[d]) + b ; feat = [relu(td),td,relu(bu),bu]
            graph pooling via one-hot matmul, indirect-scatter + AllReduce,
            FC + log_softmax computed replicated on every core.
"""

import math

import numpy as np
import ml_dtypes

import concourse.bass as bass
import concourse.bacc as bacc
import concourse.mybir as mybir
import concourse.tile as tile
from concourse.bass import IndirectOffsetOnAxis
from concourse.bass_utils import run_bass_kernel_spmd
from concourse.library_config import mlp as mlp_lib

BF16 = mybir.dt.bfloat16
F32 = mybir.dt.float32
I16 = mybir.dt.int16
I32 = mybir.dt.int32
AF = mybir.ActivationFunctionType
ALU = mybir.AluOpType
NPBF = ml_dtypes.bfloat16

P = 128  # partitions / tile height


class Cfg:
    def __init__(self, n_nodes, n_graphs, n_cores, banks, in_f, hid_f, out_f):
        assert n_nodes % n_cores == 0
        self.N = n_nodes
        self.G = n_graphs
        self.NC = n_cores
        self.NSH = n_nodes // n_cores  # nodes per core
        self.T = math.ceil(self.NSH / P)  # dst tiles per core
        self.NSH_P = self.T * P  # padded shard rows
        self.TBL = self.NC * self.NSH_P  # gather-table rows (padded global idx)
        self.BANKS = banks
        assert self.TBL % banks == 0
        self.BKSZ = self.TBL // banks
        assert self.BKSZ <= 32767, "bank local idx must fit int16"
        self.IN_F = in_f
        self.HID = hid_f
        self.FW = 2 * hid_f  # fused conv width (td|bu)
        assert self.FW == P, "kernel assumes 2*hid == 128"
        assert in_f == P, "kernel assumes in_f == 128"
        self.OUT_F = out_f
        self.FEAT = 4 * hid_f + 1  # [relu(td),td,relu(bu),bu, ones]
        self.GB = math.ceil(self.G / P)  # output graph blocks
        self.PART_ROWS = (self.G + 2 * P + P - 1) // P * P  # partial buf rows


def host_prep(cfg, x, edge_index, batch):
    """Build per-core edge grids + constants. Returns (meta, per_core_inputs)."""
    c = cfg
    src = edge_index[0].astype(np.int64)
    dst = edge_index[1].astype(np.int64)
    assert src.min() >= 0 and src.max() < c.N and dst.min() >= 0 and dst.max() < c.N

    owner = dst // c.NSH
    gidx = (src // c.NSH) * c.NSH_P + (src % c.NSH)  # padded global row of src
    bank = gidx // c.BKSZ
    lidx = (gidx % c.BKSZ).astype(np.int64)
    tloc = (dst % c.NSH) // P
    dl = ((dst % c.NSH) % P).astype(np.int64)

    ncell = c.NC * c.T * c.BANKS
    cell = (owner * c.T + tloc) * c.BANKS + bank
    order = np.argsort(cell, kind="stable")
    cell_s = cell[order]
    lidx_s = lidx[order]
    dl_s = dl[order]
    counts = np.bincount(cell_s, minlength=ncell).reshape(c.NC, c.T, c.BANKS)
    starts = np.zeros(ncell + 1, dtype=np.int64)
    np.cumsum(counts.reshape(-1), out=starts[1:])

    # chunks per (t, bank): max over cores (SPMD shares one program)
    Bmat = (-(-counts // P)).max(axis=0)  # [T, BANKS]
    GCH = int(Bmat.sum())  # total chunks per core
    ECOLS = GCH * 8  # int16 idx cols (16-wrap)

    # dummy (zero-row) pad index per bank: first pad row of the bank's first
    # core block (rows NSH..NSH_P of a block are zero in the Hn table)
    pad_lidx = np.zeros(c.BANKS, dtype=np.int64)
    for j in range(c.BANKS):
        base_block = (j * c.BKSZ) // c.NSH_P
        q = base_block * c.NSH_P + c.NSH
        if c.NSH < c.NSH_P and j * c.BKSZ <= q < (j + 1) * c.BKSZ:
            pad_lidx[j] = q - j * c.BKSZ
        else:
            pad_lidx[j] = 0  # dl=200 still excludes it from the one-hot

    g_base = np.empty(c.NC, dtype=np.int64)
    for cc in range(c.NC):
        b = batch[cc * c.NSH : (cc + 1) * c.NSH]
        g_base[cc] = int(b[0])
        assert int(b[-1]) - int(b[0]) < 2 * P, "graph span exceeds 2 blocks"

    per_core = []
    for cc in range(c.NC):
        eidx = np.zeros((P, max(ECOLS, 8)), dtype=np.int16)
        dlh = np.full((P, max(GCH, 1)), 200.0, dtype=np.float32)
        col = 0
        for t in range(c.T):
            for j in range(c.BANKS):
                B = int(Bmat[t, j])
                if B == 0:
                    continue
                ci = (cc * c.T + t) * c.BANKS + j
                s0, s1 = starts[ci], starts[ci + 1]
                n = int(s1 - s0)
                slots = B * P
                li = np.full(slots, pad_lidx[j], dtype=np.int64)
                li[:n] = lidx_s[s0:s1]
                dv = np.full(slots, 200.0, dtype=np.float64)
                dv[:n] = dl_s[s0:s1]
                # idx 16-wrap: logical i -> [i % 16, i // 16], replicated x8
                w = li.reshape(slots // 16, 16).T.astype(np.int16)  # [16, B*8]
                eidx[:, col * 8 : col * 8 + B * 8] = np.tile(w, (8, 1))
                # dl: slot i -> [i % 128, i // 128]
                dlh[:, col : col + B] = dv.reshape(B, P).T.astype(np.float32)
                col += B
        assert col == GCH

        xs = np.zeros((c.NSH_P, c.IN_F), dtype=np.float32)
        xs[: c.NSH] = x[cc * c.NSH : (cc + 1) * c.NSH]

        brel = np.full(c.T * P, 60000.0, dtype=np.float32)
        brel[: c.NSH] = batch[cc * c.NSH : (cc + 1) * c.NSH] - g_base[cc]
        batchT = brel.reshape(c.T, P).T.astype(np.float32)  # [128, T]

        goff0 = (g_base[cc] + np.arange(P)).astype(np.int32).reshape(P, 1)
        goff1 = goff0 + P
        per_core.append(
            dict(x_sh=xs, eidx=eidx, dlh=dlh, batchT=batchT, goff0=goff0, goff1=goff1)
        )

    iota_r = np.tile(np.arange(P, dtype=np.float32), (P, 1)).astype(NPBF)
    iota2_r = np.tile(np.arange(P, 2 * P, dtype=np.float32), (P, 1)).astype(NPBF)
    ident = np.eye(P, dtype=np.float32).astype(NPBF)
    consts = dict(iota_r=iota_r, iota2_r=iota2_r, ident=ident)
    meta = dict(Bmat=Bmat, GCH=GCH, ECOLS=max(ECOLS, 8), consts=consts)
    return meta, per_core


def build_program(cfg, meta, debug=False):
    c = cfg
    Bmat = meta["Bmat"]
    GCH = meta["GCH"]
    ECOLS = meta["ECOLS"]
    H = c.HID

    nc = bacc.Bacc(
        "TRN2", target_bir_lowering=False, debug=debug, num_devices=c.NC
    )

    # ---- I/O ----
    x_sh = nc.dram_tensor("x_sh", [c.NSH_P, c.IN_F], F32, kind="ExternalInput")
    W_td = nc.dram_tensor("W_td", [c.IN_F, H], F32, kind="ExternalInput")
    W_bu = nc.dram_tensor("W_bu", [c.IN_F, H], F32, kind="ExternalInput")
    b_td = nc.dram_tensor("b_td", [H], F32, kind="ExternalInput")
    b_bu = nc.dram_tensor("b_bu", [H], F32, kind="ExternalInput")
    fc_W = nc.dram_tensor("fc_W", [4 * H, c.OUT_F], F32, kind="ExternalInput")
    fc_b = nc.dram_tensor("fc_b", [c.OUT_F], F32, kind="ExternalInput")
    eidx = nc.dram_tensor("eidx", [P, ECOLS], I16, kind="ExternalInput")
    dlh = nc.dram_tensor("dlh", [P, max(GCH, 1)], F32, kind="ExternalInput")
    batchT = nc.dram_tensor("batchT", [P, c.T], F32, kind="ExternalInput")
    goff0 = nc.dram_tensor("goff0", [P, 1], I32, kind="ExternalInput")
    goff1 = nc.dram_tensor("goff1", [P, 1], I32, kind="ExternalInput")
    iota_r = nc.dram_tensor("iota_r", [P, P], BF16, kind="ExternalInput")
    iota2_r = nc.dram_tensor("iota2_r", [P, P], BF16, kind="ExternalInput")
    ident_in = nc.dram_tensor("ident", [P, P], BF16, kind="ExternalInput")
    out = nc.dram_tensor("out", [c.G, c.OUT_F], F32, kind="ExternalOutput")

    # ---- internal DRAM ----
    hn_local = nc.dram_tensor("hn_local", [c.NSH_P, c.FW], BF16)
    hn_full = nc.dram_tensor("hn_full", [c.TBL, c.FW], BF16, addr_space="Shared")
    partial = nc.dram_tensor("partial", [c.PART_ROWS, c.FEAT], F32)
    total = nc.dram_tensor("total", [c.PART_ROWS, c.FEAT], F32, addr_space="Shared")

    groups = [list(range(c.NC))]

    # tile -> (chunk col base, B) per bank
    tile_cols = []
    col = 0
    for t in range(c.T):
        bl = []
        for j in range(c.BANKS):
            B = int(Bmat[t, j])
            bl.append((col, B))
            col += B
        tile_cols.append(bl)

    with tile.TileContext(nc) as tc:
        with (
            tc.tile_pool(name="const", bufs=1) as cp,
            tc.tile_pool(name="sb", bufs=3) as sp,
        ):
            nc.gpsimd.load_library(mlp_lib)

            # ---- constants ----
            iota_sb = cp.tile([P, P], BF16)
            iota2_sb = cp.tile([P, P], BF16)
            ident_sb = cp.tile([P, P], BF16)
            nc.sync.dma_start(iota_sb[:], iota_r[:])
            nc.sync.dma_start(iota2_sb[:], iota2_r[:])
            nc.sync.dma_start(ident_sb[:], ident_in[:])
            ident32_sb = cp.tile([P, P], F32)
            nc.gpsimd.dma_start(ident32_sb[:], ident_in[:])

            wcat = cp.tile([P, c.FW], BF16)
            nc.gpsimd.dma_start(wcat[:, 0:H], W_td[:])
            nc.gpsimd.dma_start(wcat[:, H : 2 * H], W_bu[:])

            ones_bf = cp.tile([P, 1], BF16)
            nc.vector.memset(ones_bf[:], 1.0)

            bcat = cp.tile([1, c.FW], BF16)
            nc.gpsimd.dma_start(bcat[0:1, 0:H], b_td[None, :])
            nc.gpsimd.dma_start(bcat[0:1, H : 2 * H], b_bu[None, :])
            ones_row = cp.tile([1, P], BF16)
            nc.vector.memset(ones_row[:], 1.0)
            bias_sb = cp.tile([P, c.FW], F32)

            fw0 = cp.tile([P, c.OUT_F], F32)
            fw1 = cp.tile([P, c.OUT_F], F32)
            nc.sync.dma_start(fw0[:], fc_W[0:P, :])
            nc.sync.dma_start(fw1[:], fc_W[P : 2 * P, :])
            fcb = cp.tile([c.OUT_F, 1], F32)
            nc.sync.dma_start(fcb[:, 0:1], fc_b[:, None])

            eidx_sb = cp.tile([P, ECOLS], I16)
            nc.sync.dma_start(eidx_sb[:], eidx[:])
            dl_sb = cp.tile([P, max(GCH, 1)], F32)
            nc.sync.dma_start(dl_sb[:], dlh[:])
            batch_sb = cp.tile([P, c.T], F32)
            nc.sync.dma_start(batch_sb[:], batchT[:])
            goff0_sb = cp.tile([P, 1], I32)
            goff1_sb = cp.tile([P, 1], I32)
            nc.sync.dma_start(goff0_sb[:], goff0[:])
            nc.sync.dma_start(goff1_sb[:], goff1[:])

            dinv_sb = cp.tile([P, c.T], F32)

            # ---- P0b/P1/P2: bias tile, degree counts -> dinv, Hn ----
            with tc.tile_pool(name="ps12", bufs=2, space="PSUM") as pp:
                bias_ps = pp.tile([P, c.FW], F32, space="PSUM", tag="bias")
                nc.tensor.matmul(
                    bias_ps[:], lhsT=ones_row[0:1, :], rhs=bcat[0:1, :],
                    start=True, stop=True,
                )
                nc.vector.tensor_copy(bias_sb[:], bias_ps[:])

                for t in range(c.T):
                    gtc = [
                        (j, cb, B) for j, (cb, B) in enumerate(tile_cols[t]) if B
                    ]
                    nch = sum(B for _, _, B in gtc)
                    if nch == 0:
                        nc.vector.memset(dinv_sb[:, t : t + 1], 1.0)
                        continue
                    deg_ps = pp.tile([P, 1], F32, space="PSUM", tag="deg")
                    k = 0
                    for j, cb, B in gtc:
                        for q in range(B):
                            oh = sp.tile([P, P], BF16, tag="oh1")
                            nc.vector.tensor_scalar(
                                out=oh[:],
                                in0=iota_sb[:],
                                scalar1=dl_sb[:, cb + q : cb + q + 1],
                                scalar2=None,
                                op0=ALU.is_equal,
                            )
                            nc.tensor.matmul(
                                deg_ps[:],
                                lhsT=oh[:],
                                rhs=ones_bf[:],
                                start=(k == 0),
                                stop=(k == nch - 1),
                            )
                            k += 1
                    nc.scalar.activation(
                        dinv_sb[:, t : t + 1], deg_ps[:], AF.Sqrt,
                        bias=1.0, scale=1.0,
                    )
                    nc.vector.reciprocal(
                        dinv_sb[:, t : t + 1], dinv_sb[:, t : t + 1]
                    )

                for t in range(c.T):
                    xt = sp.tile([P, P], BF16, tag="xt")
                    nc.gpsimd.dma_start(xt[:], x_sh[t * P : (t + 1) * P, :])
                    xT_ps = pp.tile([P, P], BF16, space="PSUM", tag="xT")
                    nc.tensor.transpose(xT_ps[:], xt[:], ident_sb[:])
                    xT = sp.tile([P, P], BF16, tag="xTs")
                    nc.vector.tensor_copy(xT[:], xT_ps[:])
                    h_ps = pp.tile([P, c.FW], F32, space="PSUM", tag="h")
                    nc.tensor.matmul(
                        h_ps[:], lhsT=xT[:], rhs=wcat[:], start=True, stop=True
                    )
                    hn = sp.tile([P, c.FW], BF16, tag="hn")
                    nc.vector.tensor_scalar(
                        out=hn[:], in0=h_ps[:], scalar1=dinv_sb[:, t : t + 1],
                        scalar2=None, op0=ALU.mult,
                    )
                    nc.sync.dma_start(hn_local[t * P : (t + 1) * P, :], hn[:])

            # ---- P3: AllGather Hn ----
            nc.gpsimd.collective_compute(
                "AllGather",
                ALU.bypass,
                ins=[hn_local[:]],
                outs=[hn_full[:]],
                replica_groups=groups,
            )

            # ---- P4: gather + scatter-add + feat + pooling ----
            with (
                tc.tile_pool(name="gat", bufs=8) as gp,
                tc.tile_pool(name="ps4", bufs=2, space="PSUM") as pp,
                tc.tile_pool(name="psacc", bufs=1, space="PSUM") as pa,
            ):
                pool_ps0 = pa.tile([P, c.FEAT], F32, space="PSUM")
                pool_ps1 = pa.tile([P, c.FEAT], F32, space="PSUM")
                for t in range(c.T):
                    gtc = [
                        (j, cb, B) for j, (cb, B) in enumerate(tile_cols[t]) if B
                    ]
                    nch = sum(B for _, _, B in gtc)
                    gts = {}
                    for j, cb, B in gtc:
                        gt = gp.tile([P, B * P], BF16, tag="gt")
                        gts[j] = gt
                        nc.gpsimd.dma_gather(
                            gt[:].rearrange("p (b e) -> p b e", e=P),
                            hn_full[j * c.BKSZ : (j + 1) * c.BKSZ, :],
                            eidx_sb[:, cb * 8 : cb * 8 + B * 8],
                            B * P,
                            B * P,
                            c.FW,
                        )
                    acc = pp.tile([P, c.FW], F32, space="PSUM", tag="acc")
                    k = 0
                    for j, cb, B in gtc:
                        for q in range(B):
                            oh = sp.tile([P, P], BF16, tag="oh2")
                            nc.vector.tensor_scalar(
                                out=oh[:],
                                in0=iota_sb[:],
                                scalar1=dl_sb[:, cb + q : cb + q + 1],
                                scalar2=None,
                                op0=ALU.is_equal,
                            )
                            nc.tensor.matmul(
                                acc[:],
                                lhsT=oh[:],
                                rhs=gts[j][:, q * P : (q + 1) * P],
                                start=(k == 0),
                                stop=False,
                            )
                            k += 1
                    hno = sp.tile([P, c.FW], BF16, tag="hno")
                    nc.sync.dma_start(hno[:], hn_local[t * P : (t + 1) * P, :])
                    nc.tensor.matmul(
                        acc[:], lhsT=ident_sb[:], rhs=hno[:],
                        start=(nch == 0), stop=True,
                    )

                    ot = sp.tile([P, c.FW], F32, tag="ot")
                    nc.vector.tensor_scalar(
                        out=ot[:], in0=acc[:], scalar1=dinv_sb[:, t : t + 1],
                        scalar2=None, op0=ALU.mult,
                    )
                    nc.vector.tensor_tensor(
                        out=ot[:], in0=ot[:], in1=bias_sb[:], op=ALU.add
                    )
                    feat = sp.tile([P, c.FEAT], BF16, tag="feat")
                    nc.scalar.activation(feat[:, 0:H], ot[:, 0:H], AF.Relu)
                    nc.vector.tensor_copy(feat[:, H : 2 * H], ot[:, 0:H])
                    nc.scalar.activation(
                        feat[:, 2 * H : 3 * H], ot[:, H : 2 * H], AF.Relu
                    )
                    nc.vector.tensor_copy(feat[:, 3 * H : 4 * H], ot[:, H : 2 * H])
                    nc.vector.memset(feat[:, 4 * H : 4 * H + 1], 1.0)

                    ohg0 = sp.tile([P, P], BF16, tag="ohg0")
                    nc.vector.tensor_scalar(
                        out=ohg0[:], in0=iota_sb[:],
                        scalar1=batch_sb[:, t : t + 1], scalar2=None,
                        op0=ALU.is_equal,
                    )
                    ohg1 = sp.tile([P, P], BF16, tag="ohg1")
                    nc.vector.tensor_scalar(
                        out=ohg1[:], in0=iota2_sb[:],
                        scalar1=batch_sb[:, t : t + 1], scalar2=None,
                        op0=ALU.is_equal,
                    )
                    nc.tensor.matmul(
                        pool_ps0[:], lhsT=ohg0[:], rhs=feat[:],
                        start=(t == 0), stop=(t == c.T - 1),
                    )
                    nc.tensor.matmul(
                        pool_ps1[:], lhsT=ohg1[:], rhs=feat[:],
                        start=(t == 0), stop=(t == c.T - 1),
                    )

                # ---- P5: zero partial, scatter local windows ----
                zt = sp.tile([P, c.FEAT], F32, tag="zt")
                nc.vector.memset(zt[:], 0.0)
                for r in range(0, c.PART_ROWS, P):
                    nc.sync.dma_start(partial[r : r + P, :], zt[:])
                pp0 = sp.tile([P, c.FEAT], F32, tag="pp0")
                nc.vector.tensor_copy(pp0[:], pool_ps0[:])
                nc.gpsimd.indirect_dma_start(
                    out=partial[:],
                    out_offset=IndirectOffsetOnAxis(ap=goff0_sb[:, 0:1], axis=0),
                    in_=pp0[:],
                    in_offset=None,
                )
                pp1 = sp.tile([P, c.FEAT], F32, tag="pp1")
                nc.vector.tensor_copy(pp1[:], pool_ps1[:])
                nc.gpsimd.indirect_dma_start(
                    out=partial[:],
                    out_offset=IndirectOffsetOnAxis(ap=goff1_sb[:, 0:1], axis=0),
                    in_=pp1[:],
                    in_offset=None,
                )

            # ---- P6: AllReduce pooled sums ----
            nc.gpsimd.collective_compute(
                "AllReduce",
                ALU.add,
                ins=[partial[:]],
                outs=[total[:]],
                replica_groups=groups,
            )

            # ---- P7: mean, FC, log_softmax (replicated) ----
            with tc.tile_pool(name="ps7", bufs=2, space="PSUM") as pp:
                for b in range(c.GB):
                    h_rows = min(P, c.G - b * P)
                    tt = sp.tile([P, c.FEAT], F32, tag="tt")
                    nc.sync.dma_start(tt[:], total[b * P : (b + 1) * P, :])
                    rec = sp.tile([P, 1], F32, tag="rec")
                    nc.vector.tensor_scalar(
                        out=rec[:], in0=tt[:, 4 * H : 4 * H + 1], scalar1=1.0,
                        scalar2=None, op0=ALU.max,
                    )
                    nc.vector.reciprocal(rec[:], rec[:])
                    mean_bf = sp.tile([P, 4 * H], F32, tag="mean")
                    nc.vector.tensor_scalar(
                        out=mean_bf[:], in0=tt[:, 0 : 4 * H], scalar1=rec[:, 0:1],
                        scalar2=None, op0=ALU.mult,
                    )
                    lg_ps = pp.tile([P, P], F32, space="PSUM", tag="lg")
                    for half in range(2):
                        tp_ps = pp.tile([P, P], F32, space="PSUM", tag="tp")
                        nc.tensor.transpose(
                            tp_ps[:], mean_bf[:, half * P : (half + 1) * P],
                            ident32_sb[:],
                        )
                        mt = sp.tile([P, P], F32, tag="mt")
                        nc.vector.tensor_copy(mt[:], tp_ps[:])
                        nc.tensor.matmul(
                            lg_ps[0 : c.OUT_F, :],
                            lhsT=(fw0 if half == 0 else fw1)[:],
                            rhs=mt[:],
                            start=(half == 0),
                            stop=(half == 1),
                        )
                    lgb = sp.tile([c.OUT_F, P], F32, tag="lgb")
                    nc.vector.tensor_scalar(
                        out=lgb[:], in0=lg_ps[0 : c.OUT_F, :],
                        scalar1=fcb[:, 0:1], scalar2=None, op0=ALU.add,
                    )
                    tr_ps = pp.tile([P, c.OUT_F], F32, space="PSUM", tag="tr")
                    nc.tensor.transpose(
                        tr_ps[:], lgb[:], ident32_sb[0 : c.OUT_F, 0 : c.OUT_F]
                    )
                    ls = sp.tile([P, c.OUT_F], F32, tag="ls")
                    nc.vector.tensor_copy(ls[:], tr_ps[:])
                    mx = sp.tile([P, 1], F32, tag="mx")
                    nc.vector.reduce_max(mx[:], ls[:], axis=mybir.AxisListType.X)
                    nc.vector.tensor_scalar(
                        out=ls[:], in0=ls[:], scalar1=mx[:, 0:1], scalar2=None,
                        op0=ALU.subtract,
                    )
                    ex = sp.tile([P, c.OUT_F], F32, tag="ex")
                    nc.scalar.activation(ex[:], ls[:], AF.Exp)
                    sm = sp.tile([P, 1], F32, tag="sm")
                    nc.vector.reduce_sum(sm[:], ex[:], axis=mybir.AxisListType.X)
                    nc.scalar.activation(sm[:], sm[:], AF.Ln)
                    nc.vector.tensor_scalar(
                        out=ls[:], in0=ls[:], scalar1=sm[:, 0:1], scalar2=None,
                        op0=ALU.subtract,
                    )
                    nc.sync.dma_start(
                        out[b * P : b * P + h_rows, :], ls[0:h_rows, :]
                    )

    nc.compile()
    return nc


def make_in_maps(cfg, meta, per_core, W_td, b_td, W_bu, b_bu, fc_W, fc_b):
    cst = meta["consts"]
    in_maps = []
    for cc in range(cfg.NC):
        pc = per_core[cc]
        in_maps.append(
            {
                "x_sh": pc["x_sh"],
                "W_td": np.asarray(W_td, dtype=np.float32),
                "W_bu": np.asarray(W_bu, dtype=np.float32),
                "b_td": np.asarray(b_td, dtype=np.float32),
                "b_bu": np.asarray(b_bu, dtype=np.float32),
                "fc_W": np.asarray(fc_W, dtype=np.float32),
                "fc_b": np.asarray(fc_b, dtype=np.float32),
                "eidx": pc["eidx"],
                "dlh": pc["dlh"],
                "batchT": pc["batchT"],
                "goff0": pc["goff0"],
                "goff1": pc["goff1"],
                "iota_r": cst["iota_r"],
                "iota2_r": cst["iota2_r"],
                "ident": cst["ident"],
            }
        )
    return in_maps


def prep_and_build(cfg, inputs, debug=False):
    x = np.asarray(inputs["x"], dtype=np.float32)
    edge_index = np.asarray(inputs["edge_index"])
    batch = np.asarray(inputs["batch"]).astype(np.int64)
    meta, per_core = host_prep(cfg, x, edge_index, batch)
    nc = build_program(cfg, meta, debug=debug)
    in_maps = make_in_maps(
        cfg, meta, per_core,
        inputs["W_td"], inputs["b_td"], inputs["W_bu"], inputs["b_bu"],
        inputs["fc_W"], inputs["fc_b"],
    )
    return nc, in_maps


def run(cfg, inputs, debug=False, trace=False):
    nc, in_maps = prep_and_build(cfg, inputs, debug=debug)
    res = run_bass_kernel_spmd(nc, in_maps, list(range(cfg.NC)), trace=trace)
    out = res.results[0]["out"].astype(np.float32)
    return out, res


def full_cfg():
    return Cfg(
        n_nodes=100000, n_graphs=1000, n_cores=8, banks=4,
        in_f=128, hid_f=64, out_f=4,
    )


def kernel(**inputs):
    out, _ = run(full_cfg(), inputs)
    return out


# revision 9
# speedup vs baseline: 1.0731x; 1.0731x over previous
"""BiGCN (two fused GCNConv + graph mean-pool + FC + log_softmax) on 8 trn2 cores.

Strategy (graph/data parallel, partitioned by destination node range):
  - core c owns nodes [c*NSH, (c+1)*NSH) as edge destinations
  - host sorts/pads edges into per-(dst-tile, table-bank) chunks of 128
  - device: degree count via one-hot matmul -> dinv = 1/sqrt(deg+1)
            Hn = (x @ [W_td | W_bu]) * dinv  (bf16), AllGather -> full table
            dma_gather Hn[src] rows (batched over super-tiles of dst tiles),
            one-hot matmul scatter-add into PSUM
            out[d] = dinv[d]*(sum + Hn[d]) + b ; feat = [relu(td),td,relu(bu),bu]
            graph pooling via one-hot matmul, indirect-scatter + AllReduce,
            FC + log_softmax computed replicated on every core.
  - one-hot matrices are built in one DVE tensor_tensor per tile using
    broadcast access patterns (iota row vs per-slot dst-local values).
"""

import math

import numpy as np
import ml_dtypes

import concourse.bass as bass
import concourse.bacc as bacc
import concourse.mybir as mybir
import concourse.tile as tile
from concourse.bass import IndirectOffsetOnAxis
from concourse.bass_utils import run_bass_kernel_spmd
from concourse.library_config import mlp as mlp_lib

BF16 = mybir.dt.bfloat16
F32 = mybir.dt.float32
I16 = mybir.dt.int16
I32 = mybir.dt.int32
AF = mybir.ActivationFunctionType
ALU = mybir.AluOpType
NPBF = ml_dtypes.bfloat16

P = 128  # partitions / tile height


class Cfg:
    def __init__(self, n_nodes, n_graphs, n_cores, banks, in_f, hid_f, out_f,
                 sup=8):
        assert n_nodes % n_cores == 0
        self.N = n_nodes
        self.G = n_graphs
        self.NC = n_cores
        self.NSH = n_nodes // n_cores  # nodes per core
        self.T = math.ceil(self.NSH / P)  # dst tiles per core
        self.NSH_P = self.T * P  # padded shard rows
        self.TBL = self.NC * self.NSH_P  # gather-table rows
        self.BANKS = banks
        assert self.TBL % banks == 0
        self.BKSZ = self.TBL // banks
        assert self.BKSZ <= 32767, "bank local idx must fit int16"
        self.IN_F = in_f
        self.HID = hid_f
        self.FW = 2 * hid_f  # fused conv width (td|bu)
        assert self.FW == P and in_f == P
        self.OUT_F = out_f
        self.FEAT = 4 * hid_f + 1  # [relu(td),td,relu(bu),bu, ones]
        self.GB = math.ceil(self.G / P)  # output graph blocks
        self.PART_ROWS = (self.G + 2 * P + P - 1) // P * P
        self.SUP = sup  # dst tiles per gather batch
        self.NS = math.ceil(self.T / sup)  # super groups


def host_prep(cfg, x, edge_index, batch):
    """Build per-core edge grids + constants. Returns (meta, per_core_inputs)."""
    c = cfg
    src = edge_index[0].astype(np.int64)
    dst = edge_index[1].astype(np.int64)
    assert src.min() >= 0 and src.max() < c.N and dst.min() >= 0 and dst.max() < c.N

    owner = dst // c.NSH
    gidx = (src // c.NSH) * c.NSH_P + (src % c.NSH)  # padded global row of src
    bank = gidx // c.BKSZ
    lidx = (gidx % c.BKSZ).astype(np.int64)
    tloc = (dst % c.NSH) // P
    dl = ((dst % c.NSH) % P).astype(np.int64)

    ncell = c.NC * c.T * c.BANKS
    cell = (owner * c.T + tloc) * c.BANKS + bank
    order = np.argsort(cell, kind="stable")
    cell_s = cell[order]
    lidx_s = lidx[order]
    dl_s = dl[order]
    counts = np.bincount(cell_s, minlength=ncell).reshape(c.NC, c.T, c.BANKS)
    starts = np.zeros(ncell + 1, dtype=np.int64)
    np.cumsum(counts.reshape(-1), out=starts[1:])

    # chunks per (t, bank): max over cores (SPMD shares one program)
    Bmat = (-(-counts // P)).max(axis=0)  # [T, BANKS]
    GCH = int(Bmat.sum())  # total chunks per core
    ECOLS = GCH * 8  # int16 idx cols (16-wrap)

    # dummy (zero-row) pad index per bank: first pad row of the bank's first
    # core block (rows NSH..NSH_P of a block are zero in the Hn table)
    pad_lidx = np.zeros(c.BANKS, dtype=np.int64)
    for j in range(c.BANKS):
        base_block = (j * c.BKSZ) // c.NSH_P
        q = base_block * c.NSH_P + c.NSH
        if c.NSH < c.NSH_P and j * c.BKSZ <= q < (j + 1) * c.BKSZ:
            pad_lidx[j] = q - j * c.BKSZ
        else:
            pad_lidx[j] = 0  # dl=200 still excludes it from the one-hot

    g_base = np.empty(c.NC, dtype=np.int64)
    for cc in range(c.NC):
        b = batch[cc * c.NSH : (cc + 1) * c.NSH]
        g_base[cc] = int(b[0])
        assert int(b[-1]) - int(b[0]) < 2 * P, "graph span exceeds 2 blocks"

    # dl (one-hot) chunk columns ordered (t, j, q); eidx chunk columns
    # ordered (s, j, t, q) so each (super, bank) gather reads one idx slice.
    dl_col = {}
    col = 0
    for t in range(c.T):
        for j in range(c.BANKS):
            dl_col[(t, j)] = col
            col += int(Bmat[t, j])
    e_col = {}  # (s, j) -> (chunk col base, Bsum, {t: chunk offset})
    col = 0
    for s in range(c.NS):
        ts = range(s * c.SUP, min((s + 1) * c.SUP, c.T))
        for j in range(c.BANKS):
            offs = {}
            o = 0
            for t in ts:
                offs[t] = o
                o += int(Bmat[t, j])
            e_col[(s, j)] = (col, o, offs)
            col += o
    assert col == GCH

    per_core = []
    for cc in range(c.NC):
        eidx = np.zeros((P, max(ECOLS, 8)), dtype=np.int16)
        dlh = np.full((P, max(GCH, 1)), 200.0, dtype=np.float32)
        for t in range(c.T):
            s_of = t // c.SUP
            for j in range(c.BANKS):
                B = int(Bmat[t, j])
                if B == 0:
                    continue
                ci = (cc * c.T + t) * c.BANKS + j
                s0, s1 = starts[ci], starts[ci + 1]
                n = int(s1 - s0)
                slots = B * P
                li = np.full(slots, pad_lidx[j], dtype=np.int64)
                li[:n] = lidx_s[s0:s1]
                dv = np.full(slots, 200.0, dtype=np.float64)
                dv[:n] = dl_s[s0:s1]
                # idx 16-wrap: logical i -> [i % 16, i // 16], replicated x8
                w = li.reshape(slots // 16, 16).T.astype(np.int16)
                ecb = e_col[(s_of, j)][0] + e_col[(s_of, j)][2][t]
                eidx[:, ecb * 8 : ecb * 8 + B * 8] = np.tile(w, (8, 1))
                dcb = dl_col[(t, j)]
                dlh[:, dcb : dcb + B] = dv.reshape(B, P).T.astype(np.float32)

        xs = np.zeros((c.NSH_P, c.IN_F), dtype=np.float32)
        xs[: c.NSH] = x[cc * c.NSH : (cc + 1) * c.NSH]

        brel = np.full(c.T * P, 60000.0, dtype=np.float32)
        brel[: c.NSH] = batch[cc * c.NSH : (cc + 1) * c.NSH] - g_base[cc]
        batchT = brel.reshape(c.T, P).T.astype(np.float32)  # [128, T]

        goff0 = (g_base[cc] + np.arange(P)).astype(np.int32).reshape(P, 1)
        goff1 = goff0 + P
        per_core.append(
            dict(x_sh=xs, eidx=eidx, dlh=dlh.astype(NPBF), batchT=batchT,
                 goff0=goff0, goff1=goff1)
        )

    iota_r = np.tile(np.arange(P, dtype=np.float32), (P, 1)).astype(NPBF)
    iota256 = np.tile(np.arange(2 * P, dtype=np.float32), (P, 1)).astype(NPBF)
    ident = np.eye(P, dtype=np.float32).astype(NPBF)
    consts = dict(iota_r=iota_r, iota256=iota256, ident=ident)
    meta = dict(Bmat=Bmat, GCH=GCH, ECOLS=max(ECOLS, 8), consts=consts,
                dl_col=dl_col, e_col=e_col)
    return meta, per_core


def build_program(cfg, meta, debug=False):
    c = cfg
    Bmat = meta["Bmat"]
    GCH = meta["GCH"]
    ECOLS = meta["ECOLS"]
    dl_col = meta["dl_col"]
    e_col = meta["e_col"]
    H = c.HID

    nc = bacc.Bacc(
        "TRN2", target_bir_lowering=False, debug=debug, num_devices=c.NC
    )

    # ---- I/O ----
    x_sh = nc.dram_tensor("x_sh", [c.NSH_P, c.IN_F], F32, kind="ExternalInput")
    W_td = nc.dram_tensor("W_td", [c.IN_F, H], F32, kind="ExternalInput")
    W_bu = nc.dram_tensor("W_bu", [c.IN_F, H], F32, kind="ExternalInput")
    b_td = nc.dram_tensor("b_td", [H], F32, kind="ExternalInput")
    b_bu = nc.dram_tensor("b_bu", [H], F32, kind="ExternalInput")
    fc_W = nc.dram_tensor("fc_W", [4 * H, c.OUT_F], F32, kind="ExternalInput")
    fc_b = nc.dram_tensor("fc_b", [c.OUT_F], F32, kind="ExternalInput")
    eidx = nc.dram_tensor("eidx", [P, ECOLS], I16, kind="ExternalInput")
    dlh = nc.dram_tensor("dlh", [P, max(GCH, 1)], BF16, kind="ExternalInput")
    batchT = nc.dram_tensor("batchT", [P, c.T], F32, kind="ExternalInput")
    goff0 = nc.dram_tensor("goff0", [P, 1], I32, kind="ExternalInput")
    goff1 = nc.dram_tensor("goff1", [P, 1], I32, kind="ExternalInput")
    iota_r = nc.dram_tensor("iota_r", [P, P], BF16, kind="ExternalInput")
    iota256_in = nc.dram_tensor("iota256", [P, 2 * P], BF16, kind="ExternalInput")
    ident_in = nc.dram_tensor("ident", [P, P], BF16, kind="ExternalInput")
    out = nc.dram_tensor("out", [c.G, c.OUT_F], F32, kind="ExternalOutput")

    # ---- internal DRAM ----
    hn_local = nc.dram_tensor("hn_local", [c.NSH_P, c.FW], BF16)
    hn_full = nc.dram_tensor("hn_full", [c.TBL, c.FW], BF16, addr_space="Shared")
    partial = nc.dram_tensor("partial", [c.PART_ROWS, c.FEAT], F32)
    total = nc.dram_tensor("total", [c.PART_ROWS, c.FEAT], F32, addr_space="Shared")

    groups = [list(range(c.NC))]

    def tile_banks(t):
        return [(j, dl_col[(t, j)], int(Bmat[t, j]))
                for j in range(c.BANKS) if Bmat[t, j]]

    with tile.TileContext(nc) as tc:
        with (
            tc.tile_pool(name="const", bufs=1) as cp,
            tc.tile_pool(name="sb", bufs=3) as sp,
            tc.tile_pool(name="ohb", bufs=2) as op_,
        ):
            nc.gpsimd.load_library(mlp_lib)

            # ---- constants ----
            iota_sb = cp.tile([P, P], BF16)
            iota256_sb = cp.tile([P, 2 * P], BF16)
            ident_sb = cp.tile([P, P], BF16)
            nc.sync.dma_start(iota_sb[:], iota_r[:])
            nc.sync.dma_start(iota256_sb[:], iota256_in[:])
            nc.sync.dma_start(ident_sb[:], ident_in[:])
            ident32_sb = cp.tile([P, P], F32)
            nc.gpsimd.dma_start(ident32_sb[:], ident_in[:])

            wcat = cp.tile([P, c.FW], BF16)
            nc.gpsimd.dma_start(wcat[:, 0:H], W_td[:])
            nc.gpsimd.dma_start(wcat[:, H : 2 * H], W_bu[:])

            ones_bf = cp.tile([P, 1], BF16)
            nc.vector.memset(ones_bf[:], 1.0)

            bcat = cp.tile([1, c.FW], BF16)
            nc.gpsimd.dma_start(bcat[0:1, 0:H], b_td[None, :])
            nc.gpsimd.dma_start(bcat[0:1, H : 2 * H], b_bu[None, :])
            ones_row = cp.tile([1, P], BF16)
            nc.vector.memset(ones_row[:], 1.0)
            bias_sb = cp.tile([P, c.FW], F32)

            fw0 = cp.tile([P, c.OUT_F], F32)
            fw1 = cp.tile([P, c.OUT_F], F32)
            nc.sync.dma_start(fw0[:], fc_W[0:P, :])
            nc.sync.dma_start(fw1[:], fc_W[P : 2 * P, :])
            fcb = cp.tile([c.OUT_F, 1], F32)
            nc.sync.dma_start(fcb[:, 0:1], fc_b[:, None])

            dl_sb = cp.tile([P, max(GCH, 1)], BF16)
            nc.sync.dma_start(dl_sb[:], dlh[:])
            batch_sb = cp.tile([P, c.T], F32)
            nc.sync.dma_start(batch_sb[:], batchT[:])
            goff0_sb = cp.tile([P, 1], I32)
            goff1_sb = cp.tile([P, 1], I32)
            nc.sync.dma_start(goff0_sb[:], goff0[:])
            nc.sync.dma_start(goff1_sb[:], goff1[:])

            dinv_sb = cp.tile([P, c.T], F32)

            def onehot_big(t, tag):
                """One DVE op building [128, G_t*128] one-hot for all of
                tile t's chunk slots (dl columns dl_col[t,0] .. +G_t)."""
                g0 = dl_col[(t, 0)]
                gt = sum(int(Bmat[t, j]) for j in range(c.BANKS))
                oh = op_.tile([P, gt * P], BF16, tag=tag)
                nc.vector.tensor_tensor(
                    out=oh[:].rearrange("p (g d) -> p g d", d=P),
                    in0=iota_sb[:].unsqueeze(1).broadcast_to([P, gt, P]),
                    in1=dl_sb[:, g0 : g0 + gt].to_broadcast([P, gt, P]),
                    op=ALU.is_equal,
                )
                return oh, g0, gt

            # ---- P0b/P1/P2: bias tile, degree counts -> dinv, Hn ----
            with tc.tile_pool(name="ps12", bufs=2, space="PSUM") as pp:
                bias_ps = pp.tile([P, c.FW], F32, space="PSUM", tag="bias")
                nc.tensor.matmul(
                    bias_ps[:], lhsT=ones_row[0:1, :], rhs=bcat[0:1, :],
                    start=True, stop=True,
                )
                nc.vector.tensor_copy(bias_sb[:], bias_ps[:])

                for t in range(c.T):
                    nch = sum(int(Bmat[t, j]) for j in range(c.BANKS))
                    if nch == 0:
                        nc.vector.memset(dinv_sb[:, t : t + 1], 1.0)
                        continue
                    oh, g0, gt = onehot_big(t, "ohb1")
                    deg_ps = pp.tile([P, 1], F32, space="PSUM", tag="deg")
                    for q in range(gt):
                        nc.tensor.matmul(
                            deg_ps[:],
                            lhsT=oh[:, q * P : (q + 1) * P],
                            rhs=ones_bf[:],
                            start=(q == 0),
                            stop=(q == gt - 1),
                        )
                    nc.scalar.activation(
                        dinv_sb[:, t : t + 1], deg_ps[:], AF.Sqrt,
                        bias=1.0, scale=1.0,
                    )
                    nc.vector.reciprocal(
                        dinv_sb[:, t : t + 1], dinv_sb[:, t : t + 1]
                    )

                for t in range(c.T):
                    xt = sp.tile([P, P], BF16, tag="xt")
                    nc.gpsimd.dma_start(xt[:], x_sh[t * P : (t + 1) * P, :])
                    xT_ps = pp.tile([P, P], BF16, space="PSUM", tag="xT")
                    nc.tensor.transpose(xT_ps[:], xt[:], ident_sb[:])
                    xT = sp.tile([P, P], BF16, tag="xTs")
                    nc.vector.tensor_copy(xT[:], xT_ps[:])
                    h_ps = pp.tile([P, c.FW], F32, space="PSUM", tag="h")
                    nc.tensor.matmul(
                        h_ps[:], lhsT=xT[:], rhs=wcat[:], start=True, stop=True
                    )
                    hn = sp.tile([P, c.FW], BF16, tag="hn")
                    nc.vector.tensor_scalar(
                        out=hn[:], in0=h_ps[:], scalar1=dinv_sb[:, t : t + 1],
                        scalar2=None, op0=ALU.mult,
                    )
                    nc.sync.dma_start(hn_local[t * P : (t + 1) * P, :], hn[:])

            # ---- P3: AllGather Hn ----
            nc.gpsimd.collective_compute(
                "AllGather",
                ALU.bypass,
                ins=[hn_local[:]],
                outs=[hn_full[:]],
                replica_groups=groups,
            )

            # ---- P4: gather + scatter-add + feat + pooling ----
            with (
                tc.tile_pool(name="gat", bufs=8) as gp,
                tc.tile_pool(name="ps4", bufs=2, space="PSUM") as pp,
                tc.tile_pool(name="psacc", bufs=1, space="PSUM") as pa,
            ):
                pool_ps0 = pa.tile([P, c.FEAT], F32, space="PSUM")
                pool_ps1 = pa.tile([P, c.FEAT], F32, space="PSUM")
                for s in range(c.NS):
                    ts = list(range(s * c.SUP, min((s + 1) * c.SUP, c.T)))
                    gts = {}
                    for j in range(c.BANKS):
                        ecb, bsum, offs = e_col[(s, j)]
                        if bsum == 0:
                            continue
                        eix = gp.tile([P, bsum * 8], I16, tag="eix")
                        nc.sync.dma_start(
                            eix[:], eidx[:, ecb * 8 : ecb * 8 + bsum * 8]
                        )
                        gt_t = gp.tile([P, bsum * P], BF16, tag="gt")
                        gts[j] = (gt_t, offs)
                        nc.gpsimd.dma_gather(
                            gt_t[:].rearrange("p (b e) -> p b e", e=P),
                            hn_full[j * c.BKSZ : (j + 1) * c.BKSZ, :],
                            eix[:],
                            bsum * P,
                            bsum * P,
                            c.FW,
                            single_packet=(bsum * P <= 1024),
                        )
                    for t in ts:
                        nch = sum(int(Bmat[t, j]) for j in range(c.BANKS))
                        acc = pp.tile([P, c.FW], F32, space="PSUM", tag="acc")
                        if nch:
                            oh, g0, gtn = onehot_big(t, "ohb2")
                            k = 0
                            for j, dcb, B in tile_banks(t):
                                gt_t, offs = gts[j]
                                for q in range(B):
                                    nc.tensor.matmul(
                                        acc[:],
                                        lhsT=oh[:, (dcb - g0 + q) * P
                                                : (dcb - g0 + q + 1) * P],
                                        rhs=gt_t[:, (offs[t] + q) * P
                                                 : (offs[t] + q + 1) * P],
                                        start=(k == 0),
                                        stop=False,
                                    )
                                    k += 1
                        hno = sp.tile([P, c.FW], BF16, tag="hno")
                        nc.sync.dma_start(
                            hno[:], hn_local[t * P : (t + 1) * P, :]
                        )
                        nc.tensor.matmul(
                            acc[:], lhsT=ident_sb[:], rhs=hno[:],
                            start=(nch == 0), stop=True,
                        )

                        ot = sp.tile([P, c.FW], F32, tag="ot")
                        nc.vector.tensor_scalar(
                            out=ot[:], in0=acc[:],
                            scalar1=dinv_sb[:, t : t + 1],
                            scalar2=None, op0=ALU.mult,
                        )
                        nc.vector.tensor_tensor(
                            out=ot[:], in0=ot[:], in1=bias_sb[:], op=ALU.add
                        )
                        feat = sp.tile([P, c.FEAT], BF16, tag="feat")
                        nc.scalar.activation(feat[:, 0:H], ot[:, 0:H], AF.Relu)
                        nc.vector.tensor_copy(feat[:, H : 2 * H], ot[:, 0:H])
                        nc.scalar.activation(
                            feat[:, 2 * H : 3 * H], ot[:, H : 2 * H], AF.Relu
                        )
                        nc.vector.tensor_copy(
                            feat[:, 3 * H : 4 * H], ot[:, H : 2 * H]
                        )
                        nc.vector.memset(feat[:, 4 * H : 4 * H + 1], 1.0)

                        ohg = sp.tile([P, 2 * P], BF16, tag="ohg")
                        nc.vector.tensor_scalar(
                            out=ohg[:], in0=iota256_sb[:],
                            scalar1=batch_sb[:, t : t + 1], scalar2=None,
                            op0=ALU.is_equal,
                        )
                        nc.tensor.matmul(
                            pool_ps0[:], lhsT=ohg[:, 0:P], rhs=feat[:],
                            start=(t == 0), stop=(t == c.T - 1),
                        )
                        nc.tensor.matmul(
                            pool_ps1[:], lhsT=ohg[:, P : 2 * P], rhs=feat[:],
                            start=(t == 0), stop=(t == c.T - 1),
                        )

                # ---- P5: zero partial, scatter local windows ----
                zt = sp.tile([P, c.FEAT], F32, tag="zt")
                nc.vector.memset(zt[:], 0.0)
                for r in range(0, c.PART_ROWS, P):
                    nc.sync.dma_start(partial[r : r + P, :], zt[:])
                pp0 = sp.tile([P, c.FEAT], F32, tag="pp0")
                nc.vector.tensor_copy(pp0[:], pool_ps0[:])
                nc.gpsimd.indirect_dma_start(
                    out=partial[:],
                    out_offset=IndirectOffsetOnAxis(ap=goff0_sb[:, 0:1], axis=0),
                    in_=pp0[:],
                    in_offset=None,
                )
                pp1 = sp.tile([P, c.FEAT], F32, tag="pp1")
                nc.vector.tensor_copy(pp1[:], pool_ps1[:])
                nc.gpsimd.indirect_dma_start(
                    out=partial[:],
                    out_offset=IndirectOffsetOnAxis(ap=goff1_sb[:, 0:1], axis=0),
                    in_=pp1[:],
                    in_offset=None,
                )

            # ---- P6: AllReduce pooled sums ----
            nc.gpsimd.collective_compute(
                "AllReduce",
                ALU.add,
                ins=[partial[:]],
                outs=[total[:]],
                replica_groups=groups,
            )

            # ---- P7: mean, FC, log_softmax (replicated) ----
            with tc.tile_pool(name="ps7", bufs=2, space="PSUM") as pp:
                for b in range(c.GB):
                    h_rows = min(P, c.G - b * P)
                    tt = sp.tile([P, c.FEAT], F32, tag="tt")
                    nc.sync.dma_start(tt[:], total[b * P : (b + 1) * P, :])
                    rec = sp.tile([P, 1], F32, tag="rec")
                    nc.vector.tensor_scalar(
                        out=rec[:], in0=tt[:, 4 * H : 4 * H + 1], scalar1=1.0,
                        scalar2=None, op0=ALU.max,
                    )
                    nc.vector.reciprocal(rec[:], rec[:])
                    mean_sb = sp.tile([P, 4 * H], F32, tag="mean")
                    nc.vector.tensor_scalar(
                        out=mean_sb[:], in0=tt[:, 0 : 4 * H],
                        scalar1=rec[:, 0:1], scalar2=None, op0=ALU.mult,
                    )
                    lg_ps = pp.tile([P, P], F32, space="PSUM", tag="lg")
                    for half in range(2):
                        tp_ps = pp.tile([P, P], F32, space="PSUM", tag="tp")
                        nc.tensor.transpose(
                            tp_ps[:], mean_sb[:, half * P : (half + 1) * P],
                            ident32_sb[:],
                        )
                        mt = sp.tile([P, P], F32, tag="mt")
                        nc.vector.tensor_copy(mt[:], tp_ps[:])
                        nc.tensor.matmul(
                            lg_ps[0 : c.OUT_F, :],
                            lhsT=(fw0 if half == 0 else fw1)[:],
                            rhs=mt[:],
                            start=(half == 0),
                            stop=(half == 1),
                        )
                    lgb = sp.tile([c.OUT_F, P], F32, tag="lgb")
                    nc.vector.tensor_scalar(
                        out=lgb[:], in0=lg_ps[0 : c.OUT_F, :],
                        scalar1=fcb[:, 0:1], scalar2=None, op0=ALU.add,
                    )
                    tr_ps = pp.tile([P, c.OUT_F], F32, space="PSUM", tag="tr")
                    nc.tensor.transpose(
                        tr_ps[:], lgb[:], ident32_sb[0 : c.OUT_F, 0 : c.OUT_F]
                    )
                    ls = sp.tile([P, c.OUT_F], F32, tag="ls")
                    nc.vector.tensor_copy(ls[:], tr_ps[:])
                    mx = sp.tile([P, 1], F32, tag="mx")
                    nc.vector.reduce_max(mx[:], ls[:], axis=mybir.AxisListType.X)
                    nc.vector.tensor_scalar(
                        out=ls[:], in0=ls[:], scalar1=mx[:, 0:1], scalar2=None,
                        op0=ALU.subtract,
                    )
                    ex = sp.tile([P, c.OUT_F], F32, tag="ex")
                    nc.scalar.activation(ex[:], ls[:], AF.Exp)
                    sm = sp.tile([P, 1], F32, tag="sm")
                    nc.vector.reduce_sum(sm[:], ex[:], axis=mybir.AxisListType.X)
                    nc.scalar.activation(sm[:], sm[:], AF.Ln)
                    nc.vector.tensor_scalar(
                        out=ls[:], in0=ls[:], scalar1=sm[:, 0:1], scalar2=None,
                        op0=ALU.subtract,
                    )
                    nc.sync.dma_start(
                        out[b * P : b * P + h_rows, :], ls[0:h_rows, :]
                    )

    nc.compile()
    return nc


def make_in_maps(cfg, meta, per_core, W_td, b_td, W_bu, b_bu, fc_W, fc_b):
    cst = meta["consts"]
    in_maps = []
    for cc in range(cfg.NC):
        pc = per_core[cc]
        in_maps.append(
            {
                "x_sh": pc["x_sh"],
                "W_td": np.asarray(W_td, dtype=np.float32),
                "W_bu": np.asarray(W_bu, dtype=np.float32),
                "b_td": np.asarray(b_td, dtype=np.float32),
                "b_bu": np.asarray(b_bu, dtype=np.float32),
                "fc_W": np.asarray(fc_W, dtype=np.float32),
                "fc_b": np.asarray(fc_b, dtype=np.float32),
                "eidx": pc["eidx"],
                "dlh": pc["dlh"],
                "batchT": pc["batchT"],
                "goff0": pc["goff0"],
                "goff1": pc["goff1"],
                "iota_r": cst["iota_r"],
                "iota256": cst["iota256"],
                "ident": cst["ident"],
            }
        )
    return in_maps


def prep_and_build(cfg, inputs, debug=False):
    x = np.asarray(inputs["x"], dtype=np.float32)
    edge_index = np.asarray(inputs["edge_index"])
    batch = np.asarray(inputs["batch"]).astype(np.int64)
    meta, per_core = host_prep(cfg, x, edge_index, batch)
    nc = build_program(cfg, meta, debug=debug)
    in_maps = make_in_maps(
        cfg, meta, per_core,
        inputs["W_td"], inputs["b_td"], inputs["W_bu"], inputs["b_bu"],
        inputs["fc_W"], inputs["fc_b"],
    )
    return nc, in_maps


def run(cfg, inputs, debug=False, trace=False):
    nc, in_maps = prep_and_build(cfg, inputs, debug=debug)
    res = run_bass_kernel_spmd(nc, in_maps, list(range(cfg.NC)), trace=trace)
    out = res.results[0]["out"].astype(np.float32)
    return out, res


def full_cfg():
    return Cfg(
        n_nodes=100000, n_graphs=1000, n_cores=8, banks=4,
        in_f=128, hid_f=64, out_f=4,
    )


def kernel(**inputs):
    out, _ = run(full_cfg(), inputs)
    return out
